# revision 1
# baseline (speedup 1.0000x reference)
"""Trainium2 Bass kernel for nn_CSA (windowed conv-sparse-attention module).

Per-sample pipeline (B=8 -> 1 sample per NeuronCore, data-parallel):
  xcol   = im2col of padded x (one strided copy)  # ACT
  pooled = avgpool2x2(x) from xcol center taps    # DVE adds
  a      = attn_w @ pooled + attn_b               # PE, [l-part, 972-free]
  A      = softmax over q (groups of 9)           # ACT exp + DVE reduce/recip/mul
  u_q    = Wq @ xcol_q                            # PE
  o_p    = sum_q A[l,h,p,q] * u[l,q,(h,d)]        # DVE broadcast-mult + reduce
  oT     = transpose(o_p)  (l-part -> c-part)     # PE transpose
  out    = fold(oT) (overlap-add)                 # DVE strided adds
  y      = proj_w @ out + proj_b                  # PE
"""

import sys

import numpy as np

sys.path.insert(0, "/opt/trn_rl_repo")

import concourse.bass as bass  # noqa: E402
from concourse import bacc  # noqa: E402
import concourse.tile as tile  # noqa: E402
from concourse import mybir  # noqa: E402
from concourse.masks import make_identity  # noqa: E402

F32 = mybir.dt.float32
F32R = mybir.dt.float32r


def _r(ap):
    return ap.bitcast(F32R)
AF = mybir.ActivationFunctionType
ALU = mybir.AluOpType
AX = mybir.AxisListType

K, P, S, HEADS = 3, 1, 2, 12
B, C, H, W = 8, 384, 64, 64
HD = C // HEADS          # 32
K2 = K * K               # 9
N_ATT = K2 * K2 * HEADS  # 972
L = (H // S) * (W // S)  # 1024
NCORES = 8
NLC = 8                  # l-chunks of 128 windows (4 window-rows each)
CCH = C // 128           # 3 channel chunks
GW = W + 2               # padded slab width 66
GH = 10                  # slab rows per l-chunk: x rows [8*lc-1, 8*lc+9)


def _win_ap(tile_ap, qi):
    """Overlapping-window (im2col) view of a [128, GH*GW] slab for kernel
    row qi: dims (qj, i, j) -> slab element (2i+qi)*GW + (2j+qj)."""
    return bass.AP(
        tile_ap.tensor, tile_ap.offset + qi * GW,
        [list(tile_ap.ap[0])] + [[1, K], [2 * GW, 4], [2, 32]],
    )


def _build(dbg=False):
    nc = bacc.Bacc("TRN2", target_bir_lowering=False, debug=False)

    x_d = nc.declare_dram_parameter("x", [C, H * W], F32, isOutput=False)
    aw_d = nc.declare_dram_parameter("aw", [C, N_ATT], F32R, isOutput=False)
    ab_d = nc.declare_dram_parameter("ab", [1, N_ATT], F32, isOutput=False)
    cw_d = nc.declare_dram_parameter("cw", [C, K2 * C], F32R, isOutput=False)
    pw_d = nc.declare_dram_parameter("pw", [C, C], F32, isOutput=False)
    pb_d = nc.declare_dram_parameter("pb", [1, C], F32, isOutput=False)
    y_d = nc.declare_dram_parameter("y", [C, H * W], F32, isOutput=True)
    if dbg:
        pooled_d = nc.declare_dram_parameter("pooled_o", [C, L], F32, isOutput=True)
        att_d = nc.declare_dram_parameter("att_o", [L, N_ATT], F32, isOutput=True)
        u_d = nc.declare_dram_parameter("u_o", [L, K2 * C], F32, isOutput=True)
        opad_d = nc.declare_dram_parameter("opad_o", [C, GW * GW], F32, isOutput=True)

    xg = x_d.ap().rearrange("c (h w) -> c h w", h=H)

    with tile.TileContext(nc) as tc:
        with (
            tc.tile_pool(name="wts", bufs=1) as wpool,
            tc.tile_pool(name="xcol", bufs=1) as cpool,
            tc.tile_pool(name="small", bufs=2) as spool,
            tc.tile_pool(name="ubuf", bufs=2) as upool,
            tc.tile_pool(name="tbuf", bufs=3) as tpool,
            tc.tile_pool(name="obuf", bufs=3) as opool,
            tc.tile_pool(name="acc", bufs=1) as accpool,
            tc.tile_pool(name="ps_a", bufs=1, space="PSUM") as ps_a,
            tc.tile_pool(name="ps_u", bufs=2, space="PSUM") as ps_u,
            tc.tile_pool(name="ps_t", bufs=2, space="PSUM") as ps_t,
            tc.tile_pool(name="ps_y", bufs=2, space="PSUM") as ps_y,
        ):
            # ---- persistent weights in SBUF ----
            aw_sb = [wpool.tile([128, N_ATT], F32R, tag=f"aw{k}", name=f"aw{k}")
                     for k in range(CCH)]
            cw_sb = [wpool.tile([128, K2 * C], F32R, tag=f"cw{k}", name=f"cw{k}")
                     for k in range(CCH)]
            pw_sb = [wpool.tile([128, C], F32, tag=f"pw{k}", name=f"pw{k}")
                     for k in range(CCH)]
            ab_sb = wpool.tile([1, N_ATT], F32, tag="ab")
            pb_sb = wpool.tile([1, C], F32, tag="pb")
            ones_l = wpool.tile([1, 128], F32, tag="ones_l")
            ones_n = wpool.tile([1, 512], F32, tag="ones_n")
            ident = wpool.tile([128, 128], F32, tag="ident")
            ident_r = wpool.tile([128, 128], F32R, tag="ident_r")
            for k in range(CCH):
                csl = slice(k * 128, (k + 1) * 128)
                nc.sync.dma_start(out=aw_sb[k][:], in_=aw_d.ap()[csl, :])
                nc.sync.dma_start(out=cw_sb[k][:], in_=cw_d.ap()[csl, :])
                nc.sync.dma_start(out=pw_sb[k][:], in_=pw_d.ap()[csl, :])
            nc.sync.dma_start(out=ab_sb[:], in_=ab_d.ap())
            nc.sync.dma_start(out=pb_sb[:], in_=pb_d.ap())
            nc.gpsimd.memset(ones_l[:], 1.0)
            nc.gpsimd.memset(ones_n[:], 1.0)
            make_identity(nc, ident[:])
            nc.scalar.copy(out=ident_r[:], in_=ident[:])

            # folded output accumulator, padded 66x66 grid
            out_pad = [accpool.tile([128, GW * GW], F32, tag=f"op{k}",
                                    name=f"opad{k}") for k in range(CCH)]
            for k in range(CCH):
                nc.gpsimd.memset(out_pad[k][:], 0.0)

            # persistent double-buffered padded x slabs, zeroed once
            xts = [[accpool.tile([128, GH * GW], F32, tag=f"xt0{k}",
                                 name=f"xt0{k}") for k in range(CCH)]]
            for k in range(CCH):
                nc.gpsimd.memset(xts[0][k][:], 0.0)

            def _proj_stripe(t):
                rows = 7 if t < 9 else 1
                g0 = (1 + 7 * t) * GW
                for m in range(CCH):
                    msl = slice(m * 128, (m + 1) * 128)
                    py = ps_y.tile([128, 512], F32, tag="py", name="py")
                    pys = py[:, :rows * GW]
                    for k in range(CCH):
                        nc.tensor.matmul(
                            pys, pw_sb[k][:, msl],
                            out_pad[k][:, g0:g0 + rows * GW],
                            start=(k == 0), stop=False)
                    nc.tensor.matmul(pys, pb_sb[:, msl],
                                     ones_n[:, :rows * GW],
                                     start=False, stop=True)
                    yst = opool.tile([128, 448], F32, tag="yst", name="yst")
                    nc.scalar.copy(
                        out=yst[:, :rows * 64].rearrange(
                            "p (r w) -> p r w", r=rows),
                        in_=py[:, :rows * GW].rearrange(
                            "p (r w) -> p r w", r=rows)[:, :, 1:W + 1])
                    nc.sync.dma_start(
                        out=y_d.ap()[msl, 7 * t * 64:(7 * t + rows) * 64],
                        in_=yst[:, :rows * 64])

            # ---- main loop over l-chunks ----
            proj_done = 0
            for lc in range(NLC):
                xr0 = 8 * lc - 1
                r_lo, r_hi = max(0, xr0), min(H, 8 * lc + 9)
                xt = xts[0]
                xcol = [cpool.tile([128, K2 * 128], F32R, tag=f"xc{k}",
                                   name=f"xc{k}") for k in range(CCH)]
                for k in range(CCH):
                    dst = xt[k][:].rearrange("p (h w) -> p h w", h=GH)
                    if lc == NLC - 1:
                        # slab row 9 (x row 64) is stale from lc-2: re-zero
                        nc.gpsimd.memset(dst[:, GH - 1, :], 0.0)
                    nc.sync.dma_start(
                        out=dst[:, r_lo - xr0:r_hi - xr0, 1:W + 1],
                        in_=xg[k * 128:(k + 1) * 128, r_lo:r_hi, :],
                    )
                    # im2col: strided copies -> [128, (qi,qj,i,j)]
                    for qi in range(K):
                        nc.scalar.copy(
                            out=xcol[k][:, qi * 384:(qi + 1) * 384].rearrange(
                                "p (b i j) -> p b i j", b=K, i=4),
                            in_=_win_ap(xt[k][:], qi),
                        )
                    if dbg:
                        nc.sync.dma_start(
                            out=pooled_d.ap()[k * 128:(k + 1) * 128,
                                              lc * 128:(lc + 1) * 128],
                            in_=pk[:])

                # ---- attention scores: a[l, n] (n = h*81 + p*9 + q) ----
                pa = ps_a.tile([128, N_ATT], F32, tag="pa")
                nhalves = [slice(0, 512), slice(512, N_ATT)]  # bank-aligned
                for k in range(CCH):
                    for ti, tap in enumerate((4, 5, 7, 8)):
                        tsl = slice(tap * 128, (tap + 1) * 128)
                        for nsl in nhalves:
                            nc.tensor.matmul(
                                pa[:, nsl], xcol[k][:, tsl], aw_sb[k][:, nsl],
                                start=(k == 0 and ti == 0), stop=False)
                for nsl in nhalves:
                    nc.tensor.matmul(pa[:, nsl], ones_l[:], ab_sb[:, nsl],
                                     start=False, stop=True)

                # ---- softmax over q (groups of 9; logits tiny, skip max) ----
                att = spool.tile([128, N_ATT], F32, tag="att")
                nc.scalar.activation(out=att[:], in_=pa[:], func=AF.Exp)
                den = spool.tile([128, 108], F32, tag="den")
                nc.vector.tensor_reduce(
                    out=den[:], in_=att[:].rearrange("p (g q) -> p g q", q=K2),
                    axis=AX.X, op=ALU.add)
                nc.vector.reciprocal(out=den[:], in_=den[:])
                nc.vector.tensor_tensor(
                    out=att[:].rearrange("p (g q) -> p g q", q=K2),
                    in0=att[:].rearrange("p (g q) -> p g q", q=K2),
                    in1=den[:].unsqueeze(2).broadcast_to([128, 108, K2]),
                    op=ALU.mult)

                if dbg:
                    nc.sync.dma_start(
                        out=att_d.ap()[lc * 128:(lc + 1) * 128, :], in_=att[:])

                # ---- grouped conv: u[l, (q, h, d)] ----
                u_sb = upool.tile([128, K2 * C], F32, tag="u")
                for q in range(K2):
                    pu = ps_u.tile([128, C], F32, tag="pu")
                    for k in range(CCH):
                        nc.tensor.matmul(
                            pu[:], xcol[k][:, q * 128:(q + 1) * 128],
                            cw_sb[k][:, q * C:(q + 1) * C],
                            start=(k == 0), stop=(k == CCH - 1))
                    nc.scalar.copy(out=u_sb[:, q * C:(q + 1) * C], in_=pu[:])

                if dbg:
                    nc.sync.dma_start(
                        out=u_d.ap()[lc * 128:(lc + 1) * 128, :], in_=u_sb[:])

                # ---- apply + transpose + fold ----
                uv = u_sb[:].rearrange("p (q h d) -> p q h d", q=K2, h=HEADS)
                attv = att[:].rearrange("p (h pp q) -> p pp q h",
                                        h=HEADS, pp=K2)
                HCH = HEADS // CCH
                for p in range(K2):
                    pi, pj = p // K, p % K
                    for k in range(CCH):
                        h0 = k * HCH
                        tt = tpool.tile([128, K2 * 128], F32R, tag="tt")
                        nc.vector.tensor_tensor(
                            out=tt[:].rearrange("p (q h d) -> p q h d",
                                                q=K2, h=HCH),
                            in0=uv[:, :, h0:h0 + HCH, :],
                            in1=attv[:, p, :, h0:h0 + HCH].unsqueeze(3)
                                .broadcast_to([128, K2, HCH, HD]),
                            op=ALU.mult)
                        # q-sum fused into 9 PSUM-accumulated PE transposes
                        pt = ps_t.tile([128, 128], F32R, tag="pt")
                        for q in range(K2):
                            nc.tensor.matmul(
                                pt[:], tt[:, q * 128:(q + 1) * 128],
                                ident_r[:], is_transpose=True,
                                start=(q == 0), stop=(q == K2 - 1))
                        ptc = opool.tile([128, 128], F32, tag="ptc")
                        nc.scalar.copy(out=ptc[:], in_=pt[:].bitcast(F32))
                        og = out_pad[k][:].rearrange(
                            "p (h2 two w2 tw) -> p h2 two w2 tw",
                            h2=GW // 2, two=2, tw=2)
                        r0 = 8 * lc + pi
                        dst = og[:, (r0 // 2):(r0 // 2) + 4, r0 % 2,
                                 (pj // 2):(pj // 2) + 32, pj % 2]
                        nc.vector.tensor_tensor(
                            out=dst, in0=dst,
                            in1=ptc[:].rearrange("p (i j) -> p i j", i=4),
                            op=ALU.add)

                # projection stripes whose rows are now final
                ready = 10 if lc == NLC - 1 else min(9, (8 * lc) // 7) + 1
                while proj_done < ready:
                    _proj_stripe(proj_done)
                    proj_done += 1

            if dbg:
                for k in range(CCH):
                    nc.sync.dma_start(
                        out=opad_d.ap()[k * 128:(k + 1) * 128, :],
                        in_=out_pad[k][:])

    if not nc.is_finalized():
        nc.finalize()
    return nc


_NC_CACHE = None


def _get_nc():
    global _NC_CACHE
    if _NC_CACHE is None:
        _NC_CACHE = _build()
    return _NC_CACHE


def _prep_weights(attn_w, attn_b, conv_w, proj_w, proj_b):
    scale = (C // HEADS) ** -0.5
    aw = (attn_w.astype(np.float64) * scale * 0.25).astype(np.float32)
    aw_t = np.ascontiguousarray(aw.T)                                    # [C, 972]
    ab = (attn_b * scale).astype(np.float32).reshape(1, N_ATT)
    cw = conv_w.reshape(K2, C, C).transpose(2, 0, 1).reshape(C, K2 * C)  # [c_in, (q, c_out)]
    cw = np.ascontiguousarray(cw.astype(np.float32))
    pw_t = np.ascontiguousarray(proj_w.astype(np.float32).T)             # [c_in, c_out]
    pb = proj_b.astype(np.float32).reshape(1, C)
    return aw_t, ab, cw, pw_t, pb


def kernel(x, attn_w, attn_b, conv_w, proj_w, proj_b, _trace=False, _dbg=False):
    from concourse.bass_utils import run_bass_kernel_spmd

    x = np.asarray(x, dtype=np.float32)
    aw_t, ab, cw, pw_t, pb = _prep_weights(
        np.asarray(attn_w), np.asarray(attn_b), np.asarray(conv_w),
        np.asarray(proj_w), np.asarray(proj_b))
    in_maps = []
    for b in range(NCORES):
        in_maps.append({
            "x": np.ascontiguousarray(x[b].reshape(C, H * W)),
            "aw": aw_t, "ab": ab, "cw": cw, "pw": pw_t, "pb": pb,
        })
    nc = _build(dbg=True) if _dbg else _get_nc()
    res = run_bass_kernel_spmd(nc, in_maps, list(range(NCORES)), trace=False)
    if _trace:
        import time
        times = []
        for _ in range(4):
            t0 = time.perf_counter()
            res = run_bass_kernel_spmd(nc, in_maps, list(range(NCORES)),
                                       trace=False)
            np.asarray(res.results[0]["y"])
            times.append(time.perf_counter() - t0)
        print(f"run walls: {[f'{t*1e3:.1f}ms' for t in times]}")
        print("(wall includes ~50MB x 8 cores of axon-tunnel I/O per call)")
        try:
            from concourse.timeline_sim import TimelineSim
            tns = TimelineSim(nc, trace=False).simulate()
            print(f"TimelineSim per-core kernel estimate: {tns:.0f} ns")
        except Exception:
            pass
        print(f"HW exec time: {min(times)*1e9:.0f} ns")
    y = np.stack([res.results[b]["y"].reshape(C, H, W) for b in range(NCORES)])
    if _dbg:
        return y, res.results[0]
    return y

